# Initial kernel scaffold
#
"""Trainium2 Bass kernel for BaseModel.forgetting_norm.

Math (per batch b):
    m[t]  = mean over 514 channel*freq rows of x[b, :, t]
    mu[t] = alp[t] * mu[t-1] + (1 - alp[t]) * m[t]          (EMA over time)
    out[b, cf, t] = x[b, cf, t] / (mu[t] + 1e-10)

Mapping (pure data parallel, batch 32 -> 4 per core on 8 cores):
  - x loaded as [cf, t] tiles (contiguous DMA), 4x [128, 2000] + ragged
    [2, 2000] per batch (514 = 4*128 + 2; ragged rows of all 4 batches
    share one [8, 2000] tile).
  - channel mean via TensorE: lhsT = ones column, rhs = x tile -> PSUM
    row [1, <=512] per (batch, t-chunk), accumulated over the 5 cf blocks.
    The 1/514 scale is folded into the EMA input coefficients.
  - EMA via one VectorE tensor_tensor_scan (state = alp*state + bvec, fp32).
  - reciprocal of (mu + eps) computed in a [100, 80] relayout so the
    8-cycle/elem divide runs across 100 partitions instead of 4.
  - broadcast of the reciprocal row across 128 partitions via rank-1
    matmul (ones[1,128] stationary), divides on VectorE, store.
"""

import sys

sys.path.insert(0, "/opt/trn_rl_repo")

import numpy as np

import concourse.bass as bass
import concourse.bacc as bacc
import concourse.tile as tile
from concourse import mybir
from concourse.bass_utils import run_bass_kernel_spmd

import os

USE_F32R = os.environ.get("K_F32R", "0") == "1"
USE_GPSIMD_RAG = os.environ.get("K_GPRAG", "0") == "1"

B, C, F, T = 32, 2, 257, 2000
CF = C * F  # 514
NCORES = 8
BL = B // NCORES  # 4 batches per core
NFULL = CF // 128  # 4 full cf blocks
RAG = CF - NFULL * 128  # 2 ragged cf rows
EPS = 1e-10

# t-halves for the broadcast/divide stage (PSUM tile [128, 1000] = 2 banks)
HALVES = [(0, 1000), (1000, 2000)]
# reciprocal relayout: 2000 elems as [25, 80] so the 8-cycle/elem divide
# runs across 25 partitions instead of 1
PPB, RF = 25, 80


def _build_kernel(nc: bass.Bass, tc: tile.TileContext, ctx):
    f32 = mybir.dt.float32
    f32r = mybir.dt.float32r
    x = nc.dram_tensor("x", [BL, CF, T], f32, kind="ExternalInput").ap()
    alp4 = nc.dram_tensor("alp4", [BL, T], f32, kind="ExternalInput").ap()
    c14 = nc.dram_tensor("c14", [BL, T], f32, kind="ExternalInput").ap()
    out = nc.dram_tensor("out", [BL, CF, T], f32, kind="ExternalOutput").ap()

    consts = ctx.enter_context(tc.tile_pool(name="consts", bufs=1))
    xpool = ctx.enter_context(tc.tile_pool(name="xpool", bufs=8))
    ragp = ctx.enter_context(tc.tile_pool(name="ragp", bufs=1))
    xrp = ctx.enter_context(tc.tile_pool(name="xrp", bufs=2))
    rows = ctx.enter_context(tc.tile_pool(name="rows", bufs=2))
    mpsum = ctx.enter_context(tc.tile_pool(name="mpsum", bufs=2, space="PSUM"))
    rbcp = ctx.enter_context(tc.tile_pool(name="rbcp", bufs=2, space="PSUM"))

    ones_f32 = consts.tile([128, 1], f32)
    nc.vector.memset(ones_f32, 1.0)
    ones_col = consts.tile([128, 1], f32r)
    nc.scalar.copy(out=ones_col, in_=ones_f32)
    ones_row = consts.tile([1, 128], f32)
    nc.vector.memset(ones_row, 1.0)
    alp_sb = consts.tile([1, T], f32)
    nc.sync.dma_start(out=alp_sb, in_=alp4[0:1, :])
    c14_sb = consts.tile([1, T], f32)
    nc.sync.dma_start(out=c14_sb, in_=c14[0:1, :])

    # ---- loads ----
    xt = []
    for b in range(BL):
        tiles_b = []
        for cb in range(NFULL):
            t_ = xpool.tile([128, T], f32, tag="xt")
            nc.sync.dma_start(out=t_, in_=x[b, cb * 128 : (cb + 1) * 128, :])
            tiles_b.append(t_)
        xt.append(tiles_b)
    # per-batch ragged cf rows [2, T] (matmul operands must start at
    # partition 0, so these can't share one [8, T] tile)
    rag = []
    for b in range(BL):
        r_ = ragp.tile([RAG, T], f32, tag=f"rag{b}")
        nc.sync.dma_start(out=r_, in_=x[b, NFULL * 128 :, :])
        rag.append(r_)

    # ---- per-batch: channel sums (TensorE), EMA scan + reciprocal (VectorE) ----
    # The whole chain is kept per-batch so batch b's stores only depend on
    # batch b's loads (a joint [4, T] scan would deadlock xpool slot reuse).
    for b in range(BL):
        # channel sums -> m_sb [1, T]. matmul PSUM output must start at
        # partition 0/32/64, so each half goes to its own [1, 1000] PSUM
        # tile and is copied out on ScalarE.
        m_sb = rows.tile([1, T], f32, tag="msb")
        # FP32r rounding copies (ScalarE, otherwise idle): fp32r matmuls run
        # at 1 cycle/row on PE vs 4 for plain fp32, but the bir verifier
        # requires operands to come from a rounding instruction.
        if USE_F32R:
            xr = []
            for cb in range(NFULL):
                xr_ = xrp.tile([128, T], f32r, tag="xr")
                nc.scalar.copy(out=xr_, in_=xt[b][cb])
                xr.append(xr_)
            mean_lhs = ones_col
        else:
            xr = xt[b]
            mean_lhs = ones_f32
        for t0, t1 in HALVES:
            # [1, 1024] so each matmul output stays inside one PSUM bank
            mh = mpsum.tile([1, 1024], f32, tag="mh")
            for s, w in ((0, 512), (512, 488)):
                for cb in range(NFULL):
                    nc.tensor.matmul(
                        mh[:, s : s + w],
                        mean_lhs[:, 0:1],
                        xr[cb][:, t0 + s : t0 + s + w],
                        start=(cb == 0),
                        stop=False,
                    )
                # ragged rows stay plain fp32 (tiny; skips a rounding copy)
                nc.tensor.matmul(
                    mh[:, s : s + w],
                    ones_f32[0:RAG, 0:1],
                    rag[b][:, t0 + s : t0 + s + w],
                    start=False,
                    stop=True,
                )
            nc.scalar.copy(out=m_sb[:, t0:t1], in_=mh[:, 0:1000])

        # EMA scan: state = alp*state + (1-alp)/514 * sum
        nc.vector.tensor_mul(m_sb, m_sb, c14_sb)
        mu = rows.tile([1, T], f32, tag="mu")
        nc.vector.tensor_tensor_scan(
            mu, alp_sb, m_sb, 0.0, mybir.AluOpType.mult, mybir.AluOpType.add
        )
        nc.vector.tensor_scalar_add(mu, mu, EPS)

        # reciprocal in a [25, 80] relayout (8 cyc/elem -> use 25 lanes)
        mu128 = rows.tile([PPB, RF], f32, tag="mu128")
        nc.sync.dma_start(out=mu128, in_=mu)
        r128 = rows.tile([PPB, RF], f32, tag="r128")
        nc.vector.reciprocal(r128, mu128)
        rr = rows.tile([1, T], f32, tag="rrow")
        nc.sync.dma_start(out=rr, in_=r128)

        # ---- broadcast + divide + store for this batch ----
        for t0, t1 in HALVES:
            rbc = rbcp.tile([128, 1024], f32, tag="rbc")
            for s, w in ((0, 512), (512, 488)):
                nc.tensor.matmul(
                    rbc[:, s : s + w],
                    ones_row[0:1, :],
                    rr[:, t0 + s : t0 + s + w],
                    start=True,
                    stop=True,
                )
            for cb in range(NFULL):
                nc.vector.tensor_mul(
                    xt[b][cb][:, t0:t1], xt[b][cb][:, t0:t1], rbc[:, 0:1000]
                )
            if USE_GPSIMD_RAG:
                # ragged rows: copy reciprocal out of PSUM on ScalarE and
                # multiply on GpSimd (idle; GpSimd can't read PSUM)
                r2 = rows.tile([RAG, 1000], f32, tag="r2")
                nc.scalar.copy(out=r2, in_=rbc[0:RAG, 0:1000])
                nc.gpsimd.tensor_mul(
                    rag[b][:, t0:t1], rag[b][:, t0:t1], r2
                )
            else:
                nc.vector.tensor_mul(
                    rag[b][:, t0:t1], rag[b][:, t0:t1], rbc[0:RAG, 0:1000]
                )
        for cb in range(NFULL):
            nc.sync.dma_start(
                out=out[b, cb * 128 : (cb + 1) * 128, :], in_=xt[b][cb]
            )
        nc.sync.dma_start(out=out[b, NFULL * 128 :, :], in_=rag[b])


_NC_CACHE = None


def build_bass() -> bass.Bass:
    global _NC_CACHE
    if _NC_CACHE is not None:
        return _NC_CACHE
    import contextlib

    nc = bacc.Bacc("TRN2", debug=False, enable_asserts=True, num_devices=NCORES)
    with tile.TileContext(nc) as tc:
        with contextlib.ExitStack() as ctx:
            _build_kernel(nc, tc, ctx)
    nc.compile()  # reg alloc + event-semaphore wait splitting (1 wait/inst HW limit)
    _NC_CACHE = nc
    return nc


def host_coeffs(sample_length: int):
    """alp[t] exactly as the reference computes it (fp32 ops), plus the
    folded EMA input coefficient (1-alp)/CF."""
    L = int(sample_length)
    alpha = np.float32((L - 1) / (L + 1))
    idx = np.arange(T, dtype=np.float32)
    one = np.float32(1.0)
    alp = np.minimum((idx - one) / (idx + one), alpha).astype(np.float32)
    c14 = ((one - alp) / np.float32(CF)).astype(np.float32)
    alp4 = np.ascontiguousarray(np.broadcast_to(alp, (BL, T)))
    c14_4 = np.ascontiguousarray(np.broadcast_to(c14, (BL, T)))
    return alp4, c14_4


def kernel(input: np.ndarray, sample_length) -> np.ndarray:
    x = np.ascontiguousarray(np.asarray(input, dtype=np.float32)).reshape(B, CF, T)
    alp4, c14_4 = host_coeffs(int(sample_length))
    in_maps = [
        {"x": x[i * BL : (i + 1) * BL], "alp4": alp4, "c14": c14_4}
        for i in range(NCORES)
    ]
    nc = build_bass()
    res = run_bass_kernel_spmd(nc, in_maps, core_ids=list(range(NCORES)))
    full = np.concatenate([r["out"] for r in res.results], axis=0)
    return full.reshape(B, C, F, T)


if __name__ == "__main__":
    rng = np.random.default_rng(0)
    x = rng.random((B, C, F, T), dtype=np.float32)
    y = kernel(x, 192)
    print(y.shape, y.dtype)



# revision 2
# speedup vs baseline: 1.1880x; 1.1880x over previous
"""Trainium2 Bass kernel for BaseModel.forgetting_norm.

Math (per batch b):
    m[t]  = mean over 514 channel*freq rows of x[b, :, t]
    mu[t] = alp[t] * mu[t-1] + (1 - alp[t]) * m[t]          (EMA over time)
    out[b, cf, t] = x[b, cf, t] / (mu[t] + 1e-10)

Mapping (pure data parallel, batch 32 -> 4 per core on 8 cores). The
problem is HBM-bound (16.4 MB in + 16.4 MB out per core ~ 92 us at
358 GB/s), so everything else is arranged to hide under the DMA:
  - x loaded with an fp32->bf16 cast during the DMA (SWDGE), one
    [128, 4x2000] tile per batch (rows 4p..4p+3 on partition p, 32 KB
    contiguous per partition) + a ragged [2, 2000] tile (514 = 4*128+2).
    rel tolerance is 2e-2; bf16 quantization costs ~0.2%.
  - channel sums via TensorE in bf16 (1 cyc/col vs 4 for fp32):
    ones[128,1] x chunk -> PSUM [1,500] per (t-chunk), accumulated over
    the 4 row-groups + ragged rows. (1-alp)/514 is folded in afterwards.
  - EMA via one VectorE tensor_tensor_scan per batch (fp32 state).
  - reciprocal via reciprocal_approx_fast (custom DVE op, ~1 cyc/elem,
    no [25,80] relayout DMAs needed).
  - broadcast of the reciprocal row across 128 partitions via rank-1
    f32r matmul; ScalarE copies it PSUM -> SBUF as bf16 so the divides
    run as all-SBUF bf16 tensor_tensor (2x DVE mode).
  - stores cast bf16 -> fp32 during the DMA (SWDGE).
"""

import os
import sys

sys.path.insert(0, "/opt/trn_rl_repo")

import numpy as np

import concourse.bass as bass
import concourse.bacc as bacc
import concourse.tile as tile
from concourse import mybir
from concourse.bass_utils import run_bass_kernel_spmd

CASTLOAD = os.environ.get("K_CASTLOAD", "1") == "1"
CASTSTORE = os.environ.get("K_CASTSTORE", "1") == "1"

B, C, F, T = 32, 2, 257, 2000
CF = C * F  # 514
NCORES = 8
BL = B // NCORES  # 4 batches per core
NFULL = 4  # 512 = 128 * 4 rows in the main tile
RAG = CF - 128 * NFULL  # 2 ragged cf rows
TC = 4  # t-chunks for the mean matmuls
TCW = T // TC  # 500 (<=512 so each PSUM chunk stays in one bank)
HALVES = [(0, 1000), (1000, 2000)]  # broadcast PSUM tiles [128, 1024]


def _build_kernel(nc: bass.Bass, tc_: tile.TileContext, ctx):
    f32 = mybir.dt.float32
    f32r = mybir.dt.float32r
    bf16 = mybir.dt.bfloat16
    x = nc.dram_tensor("x", [BL, CF, T], f32, kind="ExternalInput").ap()
    alp = nc.dram_tensor("alp", [1, T], f32, kind="ExternalInput").ap()
    c14 = nc.dram_tensor("c14", [1, T], f32, kind="ExternalInput").ap()
    out = nc.dram_tensor("out", [BL, CF, T], f32, kind="ExternalOutput").ap()

    consts = ctx.enter_context(tc_.tile_pool(name="consts", bufs=1))
    xpool = ctx.enter_context(tc_.tile_pool(name="xpool", bufs=BL))
    ragp = ctx.enter_context(tc_.tile_pool(name="ragp", bufs=1))
    rows = ctx.enter_context(tc_.tile_pool(name="rows", bufs=2))
    rbc16p = ctx.enter_context(tc_.tile_pool(name="rbc16", bufs=2))
    mpsum = ctx.enter_context(tc_.tile_pool(name="mpsum", bufs=1, space="PSUM"))
    rbcp = ctx.enter_context(tc_.tile_pool(name="rbcp", bufs=2, space="PSUM"))
    if not CASTLOAD:
        xf32p = ctx.enter_context(tc_.tile_pool(name="xf32", bufs=2))
    if not CASTSTORE:
        of32p = ctx.enter_context(tc_.tile_pool(name="of32", bufs=2))

    ones_bf = consts.tile([128, 1], bf16)
    nc.vector.memset(ones_bf, 1.0)
    ones_row_f32 = consts.tile([1, 128], f32)
    nc.vector.memset(ones_row_f32, 1.0)
    ones_row = consts.tile([1, 128], f32r)
    nc.scalar.copy(out=ones_row, in_=ones_row_f32)
    alp_sb = consts.tile([1, T], f32)
    nc.sync.dma_start(out=alp_sb, in_=alp)
    c14_sb = consts.tile([1, T], f32)
    nc.sync.dma_start(out=c14_sb, in_=c14)

    # ---- loads (fp32 -> bf16 cast in the DMA) ----
    xt, rg = [], []
    for b in range(BL):
        t_ = xpool.tile([128, NFULL, T], bf16, tag="xt")
        src = x[b, 0 : 128 * NFULL, :].rearrange("(p j) t -> p j t", j=NFULL)
        if CASTLOAD:
            nc.gpsimd.dma_start(out=t_, in_=src)
        else:
            tf = xf32p.tile([128, NFULL, T], f32, tag="xf")
            nc.sync.dma_start(out=tf, in_=src)
            nc.scalar.copy(out=t_, in_=tf)
        xt.append(t_)
        r_ = ragp.tile([RAG, T], bf16, tag=f"rag{b}")
        nc.gpsimd.dma_start(out=r_, in_=x[b, 128 * NFULL :, :])
        rg.append(r_)

    for b in range(BL):
        # ---- channel sums (TensorE, bf16 -> fp32 PSUM) ----
        # mh[0, tc, s] accumulates sum over the 514 rows for t = tc*500+s;
        # each [1, 500] chunk stays inside PSUM bank tc.
        mh = mpsum.tile([1, TC, 512], f32, tag="mh")
        for j in range(NFULL):
            for t in range(TC):
                nc.tensor.matmul(
                    mh[:, t, 0:TCW],
                    ones_bf[:, 0:1],
                    xt[b][:, j, t * TCW : (t + 1) * TCW],
                    start=(j == 0),
                    stop=False,
                )
        for t in range(TC):
            nc.tensor.matmul(
                mh[:, t, 0:TCW],
                ones_bf[0:RAG, 0:1],
                rg[b][:, t * TCW : (t + 1) * TCW],
                start=False,
                stop=True,
            )

        # ---- EMA input b[t] = (1-alp[t])/514 * sum[t] (DVE, reads PSUM) ----
        m_sb = rows.tile([1, T], f32, tag="msb")
        nc.vector.tensor_mul(
            m_sb.rearrange("p (a s) -> p a s", a=TC),
            mh[:, :, 0:TCW],
            c14_sb.rearrange("p (a s) -> p a s", a=TC),
        )
        # ---- EMA scan: state = alp*state + b ----
        mu = rows.tile([1, T], f32, tag="mu")
        nc.vector.tensor_tensor_scan(
            mu, alp_sb, m_sb, 0.0, mybir.AluOpType.mult, mybir.AluOpType.add
        )
        # mu >= ~0.25 * min-mean here, so skipping the reference's +1e-10 is
        # a ~1e-10 relative difference; approx reciprocal is ~51 ULP.
        rr = rows.tile([1, T], f32, tag="rr")
        nc.vector.reciprocal_approx_fast(out=rr, in_=mu)
        rrr = rows.tile([1, T], f32r, tag="rrr")
        nc.scalar.copy(out=rrr, in_=rr)

        # ---- broadcast across 128 partitions; PSUM -> SBUF as bf16 ----
        rbc16 = rbc16p.tile([128, T], bf16, tag="rbc16")
        for t0, t1 in HALVES:
            rbc = rbcp.tile([128, 1024], f32, tag="rbc")
            for s, w in ((0, 512), (512, 488)):
                nc.tensor.matmul(
                    rbc[:, s : s + w],
                    ones_row[0:1, :],
                    rrr[:, t0 + s : t0 + s + w],
                    start=True,
                    stop=True,
                )
            nc.scalar.copy(out=rbc16[:, t0:t1], in_=rbc[:, 0:1000])

        # ---- divides (all-SBUF bf16 tensor_tensor, 2x mode) ----
        for j in range(NFULL):
            nc.vector.tensor_mul(xt[b][:, j, :], xt[b][:, j, :], rbc16)
        nc.vector.tensor_mul(rg[b], rg[b], rbc16[0:RAG, :])

        # ---- stores (bf16 -> fp32 cast in the DMA) ----
        dst = out[b, 0 : 128 * NFULL, :].rearrange("(p j) t -> p j t", j=NFULL)
        if CASTSTORE:
            nc.gpsimd.dma_start(out=dst, in_=xt[b])
        else:
            of = of32p.tile([128, NFULL, T], f32, tag="of")
            nc.scalar.copy(out=of, in_=xt[b])
            nc.sync.dma_start(out=dst, in_=of)
        nc.gpsimd.dma_start(out=out[b, 128 * NFULL :, :], in_=rg[b])


_NC_CACHE = None


def build_bass() -> bass.Bass:
    global _NC_CACHE
    if _NC_CACHE is not None:
        return _NC_CACHE
    import contextlib

    nc = bacc.Bacc("TRN2", debug=False, enable_asserts=True, num_devices=NCORES)
    with tile.TileContext(nc) as tc_:
        with contextlib.ExitStack() as ctx:
            _build_kernel(nc, tc_, ctx)
    nc.compile()
    _NC_CACHE = nc
    return nc


def host_coeffs(sample_length: int):
    """alp[t] exactly as the reference computes it (fp32 ops), plus the
    folded EMA input coefficient (1-alp)/CF."""
    L = int(sample_length)
    alpha = np.float32((L - 1) / (L + 1))
    idx = np.arange(T, dtype=np.float32)
    one = np.float32(1.0)
    alp = np.minimum((idx - one) / (idx + one), alpha).astype(np.float32)
    c14 = ((one - alp) / np.float32(CF)).astype(np.float32)
    return alp.reshape(1, T), c14.reshape(1, T)


def make_in_maps(x: np.ndarray, sample_length) -> list:
    x = np.ascontiguousarray(np.asarray(x, dtype=np.float32)).reshape(B, CF, T)
    alp, c14 = host_coeffs(int(sample_length))
    return [
        {"x": x[i * BL : (i + 1) * BL], "alp": alp, "c14": c14}
        for i in range(NCORES)
    ]


def kernel(input: np.ndarray, sample_length) -> np.ndarray:
    in_maps = make_in_maps(input, sample_length)
    nc = build_bass()
    res = run_bass_kernel_spmd(nc, in_maps, core_ids=list(range(NCORES)))
    full = np.concatenate([r["out"] for r in res.results], axis=0)
    return full.reshape(B, C, F, T)


if __name__ == "__main__":
    rng = np.random.default_rng(0)
    x = rng.random((B, C, F, T), dtype=np.float32)
    y = kernel(x, 192)
    print(y.shape, y.dtype)
